# revision 16
# baseline (speedup 1.0000x reference)
"""LoRA multi-head attention on 8 Trainium2 NeuronCores.

Sharding: data-parallel over batch (B=2) x tensor-parallel over heads
(16 heads -> 4 per core).  Core c handles batch b=c//4 and head group
g=c%4 (columns C=[256*g, 256*g+256) of the projection output).

Host prep (per weight): W_eff = W + 2.0 * B @ A  (exact LoRA fold),
and transposed activations x.T so the contraction dim lands on SBUF
partitions.  x and the QKV weights ship as bf16; attention runs bf16.

Device schedule (per core): the exp on the ACT engine is the hard
floor (128 x ~1.1us = ~143us), so the kernel is one software-pipelined
128-step loop (step = (qt, p, t)) that keeps ACT saturated from ~10us:
  - every step: concurrent row-tiled scores pair -> sc psum -> exp(ACT)
  - PV lags elastically (deep et ring) and drains into PE slack
  - K/V/Q projection rounds + out-projections are greedily interleaved
    into the remaining PE budget by due-date
  - inputs prefetched on 5 parallel DMA queues (sync/vector/gpsimd/
    tensor/scalar); outputs stored bf16 (host accumulates fp32)
PSUM: sc 2x[128,1024]=4 banks, ctx 2x[65,512]=2, pj 2x[128,512]=2.
"""

import sys

sys.path.insert(0, "/opt/trn_rl_repo")

from contextlib import ExitStack

import ml_dtypes
import numpy as np

import concourse.bass as bass
import concourse.tile as tile
from concourse import bacc, mybir
from concourse.bass_utils import run_bass_kernel_spmd

F32 = mybir.dt.float32
BF16 = mybir.dt.bfloat16

B = 2
S = 2048
D = 1024
H = 16
DK = 64
SCALING = 2.0
N_CORES = 8
CPG = 4
CSLICE = D // CPG
Exp = mybir.ActivationFunctionType.Exp
MULT = mybir.AluOpType.mult

N_STEPS = 128  # 4 qt x 2 p x 16 t
ET_RING = 20  # et ring depth (max PV lag + slack)
ET_CAP = 18  # force PV drain when backlog reaches this
# greedy per-step PE budget model (ns)
C_SC = 230
C_PV = 440
C_KP4 = 880
C_VP8 = 900
C_OP = 430
TARGET = 1114

_CACHE = {}


def _plan():
    """Static per-step schedule: for each of 128 steps, the list of
    interleaved work items (beyond the always-present scores+exp).
    Returns (steps, tail) where items are tuples:
      ('kp4'/'qp4', 'k'/'q', st, cc, half)  4 proj matmuls (+evict at end)
      ('vp4', tt, t4, half)                 4 V-proj matmuls (+evict at end)
      ('pv',)                               drain one lagged PV chunk
      ('op', qt, o)                         out-proj pair + evict + store
      ('xq', st)                            issue late xq DMAs for st

    Invariants enforced here (emission order == engine program order):
      - PV chunk c only after vp4(tt=(c%16)//4, t4=(c%16)%4) emitted
      - at most one open pj psum chain; no 'op' while a chain is open
      - PV chunk c at step s only if c <= s-3 (its exp long done)
      - et ring backlog bounded by ET_CAP (forced drains)
      - out-proj released 4 steps after its phase's norm (latency)
    """
    itemq = []  # (due, seq, item)
    seq = 0

    def add(due, item):
        nonlocal seq
        itemq.append((due, seq, item))
        seq += 1

    for half in range(2):
        add(11, ("kp4", "k", 0, 1, half))
    for half in range(2):
        add(12, ("qp4", "q", 0, 1, half))
    for st in (1, 2, 3):
        for cc in range(2):
            for half in range(2):
                add(4 * st - 2, ("kp4", "k", st, cc, half))
    for tt in range(4):
        for t4 in range(4):
            add(17 + 4 * tt, ("vp8", tt, t4))
    for st in (1, 2, 3):
        for cc in range(2):
            for half in range(2):
                add(32 * st - 3, ("qp4", "q", st, cc, half))
    itemq.sort(key=lambda x: (x[0], x[1]))

    cost = {"kp4": C_KP4, "qp4": C_KP4, "vp8": C_VP8}

    steps = [[] for _ in range(N_STEPS)]
    backlog = 0  # un-drained et tiles (PV lag)
    drained = 0  # PV chunks emitted
    vp_done = set()  # (tt, t4) with evict emitted
    op_pending = []  # (release_step, item)
    op_avail = []
    chain_open = False
    last_op = -9
    max_backlog = 0

    def pv_ok(c, s):
        if c > s - 3:
            return False
        t = c % 16
        return c // 16 > 0 or (t // 4, t % 4) in vp_done

    for s in range(N_STEPS):
        while op_pending and op_pending[0][0] <= s:
            op_avail.append(op_pending.pop(0)[1])
        load = C_SC
        # due-dated projection work (earliest due first)
        while itemq:
            due, _, item = itemq[0]
            c = cost[item[0]]
            if due <= s or load == C_SC or load + c <= TARGET:
                steps[s].append(item)
                load += c
                itemq.pop(0)
                if item[0] in ("kp4", "qp4"):
                    chain_open = item[4] == 0
                elif item[0] == "vp8":
                    chain_open = False
                    vp_done.add((item[1], item[2]))
            else:
                break
        # out-projection fill: one per step, >=2 steps apart, so the pj
        # WAR never waits on the DVE evict of the previous op
        if op_avail and not chain_open and s - last_op >= 2 and load + C_OP <= TARGET:
            steps[s].append(op_avail.pop(0))
            load += C_OP
            last_op = s
        # PV drain: elastic; forced when the et ring fills up
        while pv_ok(drained, s) and (load + C_PV <= TARGET or backlog >= ET_CAP):
            steps[s].append(("pv",))
            load += C_PV
            drained += 1
            backlog -= 1
            if drained % 32 == 0:
                qt = drained // 32 - 1
                if qt < 3:
                    for o in range(8):
                        op_pending.append((s + 2, ("op", qt, o)))
        backlog += 1
        max_backlog = max(max_backlog, backlog)

    assert not itemq, f"unscheduled items: {itemq[:4]}"
    assert not chain_open
    assert max_backlog <= ET_RING - 1, f"et ring too small: {max_backlog}"
    tail = [it for _, it in op_pending] + op_avail
    while drained < N_STEPS:
        tail.append(("pv",))
        drained += 1
    for o in range(8):
        tail.append(("op", 3, o))
    return steps, tail


def _build():
    nc = bacc.Bacc("TRN2", target_bir_lowering=False, debug=False)

    xqT = nc.declare_dram_parameter("xqT", [D, S], BF16, isOutput=False)
    xkT = nc.declare_dram_parameter("xkT", [D, S], BF16, isOutput=False)
    xvT = nc.declare_dram_parameter("xvT", [D, S], BF16, isOutput=False)
    wq = nc.declare_dram_parameter("wq", [128, 2048], BF16, isOutput=False)
    wk = nc.declare_dram_parameter("wk", [128, 2048], BF16, isOutput=False)
    wv = nc.declare_dram_parameter("wv", [128, 2048], BF16, isOutput=False)
    wo = nc.declare_dram_parameter("wo", [128, 2048], BF16, isOutput=False)
    outT = nc.declare_dram_parameter("outT", [D, S], BF16, isOutput=True)

    steps, tail = _plan()

    with tile.TileContext(nc) as tc, ExitStack() as ctx:
        const = ctx.enter_context(tc.tile_pool(name="const", bufs=1))
        expp = ctx.enter_context(tc.tile_pool(name="expp", bufs=ET_RING))
        smallp = ctx.enter_context(tc.tile_pool(name="smallp", bufs=2))
        psum = ctx.enter_context(tc.tile_pool(name="psum", bufs=2, space="PSUM"))

        # ---- resident SBUF tensors -------------------------------------
        wq_sb = const.tile([128, 8, CSLICE], BF16)
        wk_sb = const.tile([128, 8, CSLICE], BF16)
        wv_sb = const.tile([128, 8, CSLICE], BF16)
        wo_sb = const.tile([128, 2, D], BF16)
        # xk: st-major [st][i] tiles; xq st0; xv full-row tiles
        xk_t = [
            [const.tile([128, 512], BF16, name=f"xk{st}_{i}") for i in range(8)]
            for st in range(4)
        ]
        xq0_t = [const.tile([128, 512], BF16, name=f"xq0_{i}") for i in range(8)]
        xv_t = [const.tile([128, S], BF16, name=f"xv_{i}") for i in range(8)]
        xql_t = [const.tile([128, 3, 512], BF16, name=f"xql_{i}") for i in range(8)]

        kT_s = [const.tile([128, 2, 512], BF16, name=f"kT{i}") for i in range(4)]
        qT_s = [const.tile([128, 2, 512], BF16, name=f"qT{i}") for i in range(4)]
        v_s = [const.tile([128, 4, 4, DK + 1], BF16, name=f"v{i}") for i in range(4)]
        ctxT_s = [const.tile([128, 2, 512], BF16, name=f"cx{i}") for i in range(4)]

        # ---- DMA queues: pool-A semaphores (sync+scalar) round-robin in
        # ---- emission order, so any early scalar DMA splices its slow
        # ---- transfers into sync's chain.  Therefore: sync = wk+xk only;
        # ---- gpsimd (own pool) = wq+xq0+xv; scalar = late xq, whose
        # ---- chain position lands after sync's queue has drained.
        nc.sync.dma_start(wk_sb[:], wk.rearrange("p (i c) -> p i c", c=CSLICE))
        for st in range(4):
            for i in range(8):
                nc.sync.dma_start(
                    xk_t[st][i][:],
                    xkT[128 * i : 128 * (i + 1), 512 * st : 512 * (st + 1)],
                )
        nc.gpsimd.dma_start(wq_sb[:], wq.rearrange("p (i c) -> p i c", c=CSLICE))
        for i in range(8):
            nc.gpsimd.dma_start(xq0_t[i][:], xqT[128 * i : 128 * (i + 1), 0:512])
        nc.gpsimd.dma_start(wv_sb[:], wv.rearrange("p (i c) -> p i c", c=CSLICE))
        for i in range(8):
            nc.gpsimd.dma_start(xv_t[i][:], xvT[128 * i : 128 * (i + 1), :])
        nc.gpsimd.dma_start(wo_sb[:], wo.rearrange("p (j o) -> p j o", o=D))
        for i in range(8):
            nc.scalar.dma_start(xql_t[i][:], xqT[128 * i : 128 * (i + 1), 512:2048])

        ones_f = const.tile([128, 16], F32)
        nc.vector.memset(ones_f[:], 1.0)
        # PE warm-up: the HAM starts the PE at half clock and ramps only
        # after sustained activity; burn cheap matmuls on memset data so
        # the real prologue chains run at full rate.
        warm_b = const.tile([128, 256], BF16, name="warm_b")
        nc.vector.memset(warm_b[:], 1.0)
        for w in range(40):
            wps = psum.tile([128, 256], F32, tag="pj", bufs=2, name=f"wup{w}")
            nc.tensor.matmul(
                wps[:], warm_b[0:16, 0:128], warm_b[0:16, :], start=True, stop=True
            )
        for tt in range(4):
            nc.vector.tensor_copy(
                v_s[tt][:, :, :, DK : DK + 1],
                ones_f[:].rearrange("p (a b c) -> p a b c", a=4, b=4, c=1),
            )

        # ---- emit helpers ----------------------------------------------
        pj_live = {}  # (kind, st, cc) -> psum tile for open proj chains

        def proj4(kind, st, cc, half):
            wsb = wk_sb if kind == "k" else wq_sb
            dst = kT_s[st] if kind == "k" else qT_s[st]
            xts = xk_t[st] if kind == "k" else (xq0_t if st == 0 else None)
            key = (kind, st, cc)
            if half == 0:
                pj_live[key] = psum.tile([128, 512], F32, tag="pj", bufs=2, name=f"pj{st}{cc}")
            ps = pj_live[key]
            for i in range(4 * half, 4 * half + 4):
                xt = xts[i][:] if xts is not None else xql_t[i][:, st - 1, :]
                nc.tensor.matmul(
                    ps[:],
                    wsb[:, i, 128 * cc : 128 * (cc + 1)],
                    xt,
                    start=(i == 0),
                    stop=(i == 7),
                )
            if half == 1:
                del pj_live[key]
                nc.vector.tensor_copy(dst[:, cc, :], ps[:])

        def vp4(tt, t4, half):
            key = ("v", tt, t4)
            if half == 0:
                pj_live[key] = psum.tile(
                    [128, 256], F32, tag="pj", bufs=2, name=f"vps{tt}_{t4}"
                )
            ps = pj_live[key]
            for i in range(4 * half, 4 * half + 4):
                nc.tensor.matmul(
                    ps[:],
                    xv_t[i][:, 512 * tt + 128 * t4 : 512 * tt + 128 * (t4 + 1)],
                    wv_sb[:, i, :],
                    start=(i == 0),
                    stop=(i == 7),
                )
            if half == 1:
                del pj_live[key]
                nc.vector.tensor_copy(
                    v_s[tt][:, t4, :, 0:DK],
                    ps[:].rearrange("p (h d) -> p h d", h=4),
                )

        def scores(qt, p, t, s):
            sc = psum.tile([128, 1024], F32, tag="sc", bufs=2, name=f"sc{s}")
            kt = kT_s[t // 4]
            qtile = qT_s[qt]
            ts_ = slice(128 * (t % 4), 128 * (t % 4 + 1))
            nc.tensor.matmul(
                sc[:, 0:512],
                kt[0:64, p, ts_],
                qtile[0:64, p, :],
                start=True,
                stop=True,
                tile_position=(0, 0),
            )
            nc.tensor.matmul(
                sc[:, 512:1024],
                kt[64:128, p, ts_],
                qtile[64:128, p, :],
                start=True,
                stop=True,
                tile_position=(64, 0),
            )
            et = expp.tile([128, 1024], BF16, tag="et", bufs=ET_RING, name=f"et{s}")
            nc.scalar.activation(et[:], sc[:], Exp, scale=1.0 / 8.0)
            return et

        ctx_live = [None, None]

        def pv_chunk(chunk, ets):
            """PV for global chunk index (phase = chunk//16, t = chunk%16)."""
            phase, t = divmod(chunk, 16)
            qt, p = divmod(phase, 2)
            if t == 0:
                ctx_live[0] = psum.tile(
                    [DK + 1, 512], F32, tag="ctx", bufs=2, name=f"ctx0_{phase}"
                )
                ctx_live[1] = psum.tile(
                    [DK + 1, 512], F32, tag="ctx", bufs=2, name=f"ctx1_{phase}"
                )
            et = ets.pop(16 * phase + t)
            nc.tensor.matmul(
                ctx_live[0][:],
                v_s[t // 4][:, t % 4, 2 * p, :],
                et[:, 0:512],
                start=(t == 0),
                stop=(t == 15),
            )
            nc.tensor.matmul(
                ctx_live[1][:],
                v_s[t // 4][:, t % 4, 2 * p + 1, :],
                et[:, 512:1024],
                start=(t == 0),
                stop=(t == 15),
            )
            if t == 15:
                norm(qt, p)

        def norm(qt, p):
            # evict ctx psums first (fast PSUM release); then normalize with
            # the two heads interleaved to shorten the serial chain
            css, rcs, bcs, cts = [], [], [], []
            for cx in ctx_live:
                cs = smallp.tile([DK + 1, 512], F32, tag="cs", bufs=3)
                nc.vector.tensor_copy(cs[:], cx[:])
                css.append(cs)
            ctx_live[0] = ctx_live[1] = None
            for h01, cs in enumerate(css):
                rs1 = smallp.tile([1, 512], F32, tag="rs1", bufs=2)
                nc.vector.tensor_copy(rs1[:], cs[DK : DK + 1, :])
                rc = smallp.tile([1, 512], F32, tag="rc", bufs=2)
                nc.vector.reciprocal_approx_fast(rc[:], rs1[:])
                rcs.append(rc)
            for h01 in range(2):
                bc = smallp.tile([64, 512], F32, tag="bc", bufs=2)
                nc.gpsimd.partition_broadcast(bc[:], rcs[h01][:])
                bcs.append(bc)
            for h01 in range(2):
                ct = smallp.tile([64, 512], BF16, tag="ct", bufs=2)
                nc.vector.tensor_tensor(ct[:], css[h01][0:DK, :], bcs[h01][:], MULT)
                cts.append(ct)
            for h01 in range(2):
                nc.sync.dma_start(
                    ctxT_s[qt][64 * h01 : 64 * h01 + 64, p, :], cts[h01][:]
                )

        def outproj(qt, o, tail_evict=False):
            ops = psum.tile([128, 512], F32, tag="pj", bufs=2, name=f"op{qt}_{o}")
            nc.tensor.matmul(
                ops[:],
                wo_sb[:, 0, 128 * o : 128 * (o + 1)],
                ctxT_s[qt][:, 0, :],
                start=True,
                stop=False,
            )
            nc.tensor.matmul(
                ops[:],
                wo_sb[:, 1, 128 * o : 128 * (o + 1)],
                ctxT_s[qt][:, 1, :],
                start=False,
                stop=True,
            )
            ob = smallp.tile([128, 512], BF16, tag="ob", bufs=2)
            if tail_evict:
                # the ACT engine is idle after the last exp; use it for
                # every other tail eviction so the DVE isn't the choke
                nc.scalar.copy(ob[:], ops[:])
            else:
                nc.vector.tensor_copy(ob[:], ops[:])
            nc.gpsimd.dma_start(
                outT[128 * o : 128 * (o + 1), 512 * qt : 512 * (qt + 1)], ob[:]
            )

        # ---- prologue compute: KP(0) cc0, QP(0) cc0 (all that p=0
        # ---- scores need); the cc1 halves ride as step items ------------
        for half in range(2):
            proj4("k", 0, 0, half)
        for half in range(2):
            proj4("q", 0, 0, half)

        # ---- the 128-step pipelined loop -------------------------------
        ets = {}
        pv_done = 0
        for s in range(N_STEPS):
            qt, r = divmod(s, 32)
            p, t = divmod(r, 16)
            # PV + interleave first (program order), scores last would
            # delay the ACT feed; emit scores first, then fill.
            ets[s] = scores(qt, p, t, s)
            for item in steps[s]:
                kind = item[0]
                if kind == "pv":
                    pv_chunk(pv_done, ets)
                    pv_done += 1
                elif kind == "kp4":
                    proj4("k", item[2], item[3], item[4])
                elif kind == "qp4":
                    proj4("q", item[2], item[3], item[4])
                elif kind == "vp8":
                    vp4(item[1], item[2], 0)
                    vp4(item[1], item[2], 1)
                elif kind == "op":
                    outproj(item[1], item[2])
        first_op = True
        n_tail_op = 0
        for item in tail:
            if item[0] == "pv":
                pv_chunk(pv_done, ets)
                pv_done += 1
            else:
                if first_op:
                    # keep the PE at full pstate while the last norm chain
                    # drains (idle >3.4us re-throttles the clock)
                    for w in range(32):
                        warm = psum.tile(
                            [128, 512], F32, tag="pj", bufs=2, name=f"warm{w}"
                        )
                        nc.tensor.matmul(
                            warm[:],
                            wo_sb[:, 0, 0:128],
                            kT_s[3][:, 0, :],
                            start=True,
                            stop=True,
                        )
                    first_op = False
                n_tail_op += 1
                outproj(item[1], item[2], tail_evict=(n_tail_op % 2 == 0))

    nc.finalize()
    return nc


def _get_nc():
    if "nc" not in _CACHE:
        _CACHE["nc"] = _build()
    return _CACHE["nc"]


def _numpy_reference(query, key, value, mask, Wq, Aq, Bq, Wk, Ak, Bk, Wv, Av, Bv, Wo, Ao, Bo):
    """Exact fallback for a non-all-ones mask (never hit for the spec'd inputs)."""

    def lora(x, W, A, Bm):
        return x @ W.T + ((x @ A.T) @ Bm.T) * SCALING

    q = lora(query, Wq, Aq, Bq).reshape(B, S, H, DK).transpose(0, 2, 1, 3)
    k = lora(key, Wk, Ak, Bk).reshape(B, S, H, DK).transpose(0, 2, 1, 3)
    v = lora(value, Wv, Av, Bv).reshape(B, S, H, DK).transpose(0, 2, 1, 3)
    sc = np.einsum("bhqd,bhkd->bhqk", q, k) / np.sqrt(np.float32(DK))
    sc = np.where(mask == 0, np.float32(-1e9), sc)
    sc = sc - sc.max(axis=-1, keepdims=True)
    e = np.exp(sc)
    attn = e / e.sum(axis=-1, keepdims=True)
    cx = np.einsum("bhqk,bhkd->bhqd", attn, v)
    cx = cx.transpose(0, 2, 1, 3).reshape(B, S, D)
    return lora(cx, Wo, Ao, Bo).astype(np.float32)


def _warr(w, n):
    # [n*128, m] -> [128, n*m] with row (i*128+p) at [p, i*m:(i+1)*m]:
    # per-partition contiguous so the weight DMA needs no striding
    m = w.shape[1]
    return np.ascontiguousarray(
        w.reshape(n, 128, m).transpose(1, 0, 2).reshape(128, n * m)
    ).astype(ml_dtypes.bfloat16)


def _prepare_in_maps(query, key, value, Wq, Aq, Bq, Wk, Ak, Bk, Wv, Av, Bv, Wo, Ao, Bo):
    f32 = np.float32
    bf16 = ml_dtypes.bfloat16
    weff = {}
    for n, (W, A, Bm) in {
        "q": (Wq, Aq, Bq),
        "k": (Wk, Ak, Bk),
        "v": (Wv, Av, Bv),
        "o": (Wo, Ao, Bo),
    }.items():
        weff[n] = (
            np.asarray(W, f32) + SCALING * np.asarray(Bm, f32) @ np.asarray(A, f32)
        ).astype(f32)

    xT = {
        "q": [np.ascontiguousarray(np.asarray(query[b], f32).T).astype(bf16) for b in range(B)],
        "k": [np.ascontiguousarray(np.asarray(key[b], f32).T).astype(bf16) for b in range(B)],
        "v": [np.ascontiguousarray(np.asarray(value[b], f32).T).astype(bf16) for b in range(B)],
    }
    in_maps = []
    for c in range(N_CORES):
        b, g = divmod(c, CPG)
        cs = slice(CSLICE * g, CSLICE * (g + 1))
        in_maps.append(
            {
                "xqT": xT["q"][b],
                "xkT": xT["k"][b],
                "xvT": xT["v"][b],
                "wq": _warr(weff["q"][cs, :].T, 8),
                "wk": _warr(weff["k"][cs, :].T, 8),
                "wv": _warr(weff["v"][cs, :].T, 8),
                "wo": _warr(weff["o"][:, cs].T, 2),
            }
        )
    return in_maps


def run(inputs, trace=False, **spmd_kwargs):
    """Shard, run on 8 cores, gather.  Returns (output, BassKernelResults)."""
    mask = np.asarray(inputs["mask"])
    if not np.all(mask != 0):
        out = _numpy_reference(
            np.asarray(inputs["query"], np.float32),
            np.asarray(inputs["key"], np.float32),
            np.asarray(inputs["value"], np.float32),
            mask,
            *[
                np.asarray(inputs[k], np.float32)
                for k in ("Wq", "Aq", "Bq", "Wk", "Ak", "Bk", "Wv", "Av", "Bv", "Wo", "Ao", "Bo")
            ],
        )
        return out, None

    in_maps = _prepare_in_maps(
        inputs["query"], inputs["key"], inputs["value"],
        inputs["Wq"], inputs["Aq"], inputs["Bq"],
        inputs["Wk"], inputs["Ak"], inputs["Bk"],
        inputs["Wv"], inputs["Av"], inputs["Bv"],
        inputs["Wo"], inputs["Ao"], inputs["Bo"],
    )
    nc = _get_nc()
    res = run_bass_kernel_spmd(
        nc, in_maps, core_ids=list(range(N_CORES)), trace=trace, **spmd_kwargs
    )
    out = np.empty((B, S, D), np.float32)
    for b in range(B):
        acc = res.results[CPG * b]["outT"].astype(np.float32)
        for g in range(1, CPG):
            acc = acc + res.results[CPG * b + g]["outT"].astype(np.float32)
        out[b] = acc.T
    return out, res


def kernel(**inputs):
    out, _ = run(inputs, trace=False)
    return out
